# revision 4
# baseline (speedup 1.0000x reference)
"""Trainium2 Bass kernel for the 5-layer LSTM + FC head (nn_LstmMemoryPredict).

Data-parallel over 8 NeuronCores (batch 256 -> 32 per core). The T=2048
recurrence is split into P=8 overlapping segments per core (warmup K=32;
forget-gate decay makes the seam error ~1e-6). The 8 segment-chains run
round-robin, hiding the ~2.7us serial LSTM-cell dependency chain behind
engine throughput. Per chain-step a 5-layer wavefront computes all gates
with one 128-partition sigmoid (gate order [f,i,o,g]; tanh via sigma(2x)
and a per-partition ACT scale vector). x ships as fp8_e4m3, transposed
on-device by the PE. Self-contained.
"""
import sys
sys.path.insert(0, "/opt/trn_rl_repo")

import numpy as np
import concourse.bass as bass
import concourse.bacc as bacc
import concourse.mybir as mybir
from concourse.tile import TileContext
from concourse.mybir import AluOpType, ActivationFunctionType

F32 = mybir.dt.float32
F16 = mybir.dt.float16
BF16 = mybir.dt.bfloat16
F8 = mybir.dt.float8e4

H, L, D = 32, 5, 64
BC = 32                  # batch per core
F = L * BC               # 160: free size of one chain step
PRO = L - 1              # wavefront warm-up steps
T = 2048
P = 8                    # recurrence segments (parallel chains)
K = 32                   # segment state warm-up steps
N = (T + (P - 1) * K) // P   # 284 main rounds
RB = 4                   # rounds per For_i body
NBODY = N // RB          # 71 bodies
SEG = N - K              # 252: t0 stride between chains
# torch gate order (i,f,g,o) -> kernel order (f,i,o,g)
GATE_PERM = np.r_[32:64, 0:32, 96:128, 64:96]


def build(unroll=False):
    nc = bacc.Bacc(None, target_bir_lowering=False, debug=False)

    xt_d = nc.dram_tensor("xt", [BC, T * D], F8, kind="ExternalInput")
    w0_d = nc.dram_tensor("w0", [D, 128], BF16, kind="ExternalInput")
    wcat_d = nc.dram_tensor("wcat", [H, 9 * 128], BF16, kind="ExternalInput")
    b5_d = nc.dram_tensor("b5", [L, 128], BF16, kind="ExternalInput")
    oneh_d = nc.dram_tensor("oneh", [L, F], BF16, kind="ExternalInput")
    svec_d = nc.dram_tensor("svec", [128, 1], F32, kind="ExternalInput")
    fcw_d = nc.dram_tensor("fcw", [H + 1, 1], BF16, kind="ExternalInput")
    ident_d = nc.dram_tensor("ident", [BC, BC], F8, kind="ExternalInput")
    out_d = nc.dram_tensor("out", [1, N * P * BC], F32, kind="ExternalOutput")

    with TileContext(nc) as tc:
        with (
            tc.tile_pool(name="wpool", bufs=1) as wpool,
            tc.tile_pool(name="state", bufs=1) as state,
            tc.tile_pool(name="xpool", bufs=3) as xpool,
            tc.tile_pool(name="xsbp", bufs=3) as xsbp,
            tc.tile_pool(name="spool", bufs=6) as spool,
            tc.tile_pool(name="opool", bufs=3) as opool,
            tc.tile_pool(name="zpool", bufs=4, space="PSUM") as zpool,
            tc.tile_pool(name="xtp", bufs=2, space="PSUM") as xtp,
            tc.tile_pool(name="fcp", bufs=2, space="PSUM") as fcp,
        ):
            # ---- constants / weights ----
            w0 = wpool.tile([D, 128], BF16)
            wcat = wpool.tile([H, 9 * 128], BF16)
            b5 = wpool.tile([L, 128], BF16)
            oneh = wpool.tile([L, F], BF16)
            svec = wpool.tile([128, 1], F32)
            fcw = wpool.tile([H + 1, 1], BF16)
            ident = wpool.tile([BC, BC], F8)
            nc.sync.dma_start(w0[:], w0_d[:, :])
            nc.sync.dma_start(wcat[:], wcat_d[:, :])
            nc.sync.dma_start(b5[:], b5_d[:, :])
            nc.sync.dma_start(oneh[:], oneh_d[:, :])
            nc.sync.dma_start(svec[:], svec_d[:, :])
            nc.sync.dma_start(fcw[:], fcw_d[:, :])
            nc.sync.dma_start(ident[:], ident_d[:, :])

            # ---- persistent state (per chain) ----
            hst = [state.tile([H + 1, F], BF16, tag=f"hst{p}", name=f"hst{p}")
                   for p in range(P)]
            cst = [state.tile([H, F], F16, tag=f"cst{p}", name=f"cst{p}")
                   for p in range(P)]
            for p in range(P):
                nc.gpsimd.memset(hst[p][:], 0.0)
                nc.gpsimd.memset(hst[p][H:H + 1, :], 1.0)
                nc.gpsimd.memset(cst[p][:], 0.0)

            def wh(l):
                return wcat[:, (2 * l) * 128:(2 * l + 1) * 128]

            def wx(l):
                return wcat[:, (2 * l - 1) * 128:(2 * l) * 128]

            def emit_step(p, x_mv, fc_ps=None, fc_col=0, mask_from=None):
                zb = zpool.tile([128, F], F32, tag="zb", name="zb")
                nc.tensor.matmul(zb[:, :], b5[:], oneh[:], start=True,
                                 stop=False, skip_group_check=True)
                nc.tensor.matmul(zb[:, 0:BC], w0[:], x_mv, start=False,
                                 stop=False, skip_group_check=True)
                for l in range(L):
                    nc.tensor.matmul(zb[:, l * BC:(l + 1) * BC], wh(l),
                                     hst[p][0:H, l * BC:(l + 1) * BC],
                                     start=False, stop=False,
                                     skip_group_check=True)
                for l in range(1, L):
                    nc.tensor.matmul(zb[:, l * BC:(l + 1) * BC], wx(l),
                                     hst[p][0:H, (l - 1) * BC:l * BC],
                                     start=False, stop=(l == L - 1),
                                     skip_group_check=True)
                sg = spool.tile([128, F], F16, tag="sg", name="sg")
                nc.scalar.activation(sg[:, :], zb[:, :],
                                     ActivationFunctionType.Sigmoid,
                                     scale=svec[:, 0:1])
                # cell: g' = 2*sig(2zg)-1 (base 32); c' = f*c + i*g'
                a64 = spool.tile([2 * H, F], F16, tag="a64", name="a64")
                nc.vector.tensor_scalar(a64[H:2 * H, :], sg[96:128, :],
                                        2.0, -1.0, AluOpType.mult,
                                        AluOpType.add)
                nc.vector.tensor_tensor(a64[0:H, :], sg[0:H, :],
                                        cst[p][:, :], AluOpType.mult)
                p2 = spool.tile([H, F], F16, tag="p2", name="p2")
                nc.vector.tensor_tensor(p2[:, :], sg[H:2 * H, :],
                                        a64[H:2 * H, :], AluOpType.mult)
                nc.vector.tensor_tensor(cst[p][:, :], a64[0:H, :], p2[:, :],
                                        AluOpType.add)
                th = spool.tile([3 * H, F], F16, tag="th", name="th")
                nc.scalar.activation(th[2 * H:3 * H, :], cst[p][:, :],
                                     ActivationFunctionType.Tanh)
                nc.vector.tensor_tensor(hst[p][0:H, :], sg[64:96, :],
                                        th[2 * H:3 * H, :], AluOpType.mult)
                if fc_ps is not None:
                    nc.tensor.matmul(fc_ps[:, fc_col:fc_col + BC], fcw[:],
                                     hst[p][:, (L - 1) * BC:F], start=True,
                                     stop=True, skip_group_check=True)
                if mask_from is not None:
                    nc.gpsimd.memset(hst[p][0:H, mask_from * BC:F], 0.0)
                    nc.gpsimd.memset(cst[p][:, mask_from * BC:F], 0.0)

            def load_x(off_expr, nsteps, skip_chains=()):
                """DMA x for all P chains for `nsteps` rounds starting at
                x-time (t0_p + off) and transpose to [D, ...] layout.
                Returns xsb with col layout p*(nsteps*BC) + k*BC + b."""
                xb = xpool.tile([BC, P * nsteps * D], F8, name="xb")
                for p in skip_chains:
                    nc.gpsimd.memset(
                        xb[:, p * nsteps * D:(p + 1) * nsteps * D], 0.0)
                for p in range(P):
                    if p not in skip_chains:
                        nc.sync.dma_start(
                            xb[:, p * nsteps * D:(p + 1) * nsteps * D],
                            xt_d[:, bass.ds(p * SEG * D + off_expr,
                                            nsteps * D)])
                xt_ps = xtp.tile([D, P * nsteps * BC * 2], F8, tag="xt",
                                 name="xt_ps")
                for p in range(P):
                    for k in range(nsteps):
                        j = p * nsteps + k
                        nc.tensor.transpose(
                            xt_ps[:, 2 * j * BC:2 * (j + 1) * BC:2],
                            xb[:, j * D:(j + 1) * D], ident[:])
                xsb = xsbp.tile([D, P * nsteps * BC], BF16, name="xsb")
                nc.vector.tensor_copy(xsb[:, :],
                                      xt_ps[:, 0:2 * P * nsteps * BC:2])
                return xsb

            # ---- prologue: wavefront warm-up, rounds w=0..3 ----
            xsb_pro = load_x(0, PRO)
            for w in range(PRO):
                for p in range(P):
                    emit_step(p, xsb_pro[:, (p * PRO + w) * BC:
                                         (p * PRO + w + 1) * BC],
                              mask_from=w + 1)

            # ---- main loop: NBODY bodies of RB rounds ----
            def emit_body(i, last=False):
                # x for rounds i*RB .. i*RB+RB-1: x-time t0_p + PRO + i*RB + k
                if last:
                    xsb = load_x((PRO + i * RB) * D, RB, skip_chains=(P - 1,))
                else:
                    xsb = load_x((PRO + i * RB) * D
                                 if isinstance(i, int) else
                                 i * (RB * D) + PRO * D, RB)
                for half in range(RB // 2):
                    fc_ps = fcp.tile([1, 2 * P * BC], F32, tag="fc",
                                     name="fc_ps")
                    for k2 in range(2):
                        k = half * 2 + k2
                        for p in range(P):
                            emit_step(p, xsb[:, (p * RB + k) * BC:
                                             (p * RB + k + 1) * BC],
                                      fc_ps=fc_ps,
                                      fc_col=(k2 * P + p) * BC)
                    ost = opool.tile([1, 2 * P * BC], F32, tag="ost",
                                     name="ost")
                    nc.vector.tensor_copy(ost[:], fc_ps[:, :])
                    if isinstance(i, int):
                        off = (i * RB + half * 2) * (P * BC)
                    else:
                        off = i * (RB * P * BC) + half * 2 * (P * BC)
                    nc.sync.dma_start(
                        out_d[:, bass.ds(off, 2 * P * BC)], ost[:])

            if unroll:
                for i in range(NBODY - 1):
                    emit_body(i)
            else:
                with tc.For_i(0, NBODY - 1) as i:
                    emit_body(i)
            emit_body(NBODY - 1, last=True)

    nc.compile()
    return nc


# ---------------- host-side packing ----------------

def prep_weights(W_ih0, W_ih_rest, W_hh, b_ih, b_hh, W_fc, b_fc):
    import ml_dtypes
    bf = ml_dtypes.bfloat16
    p = GATE_PERM
    w0 = np.ascontiguousarray(W_ih0[p].T).astype(bf)                # [64,128]
    blocks = [W_hh[0][p].T]
    for l in range(1, L):
        blocks.append(W_ih_rest[l - 1][p].T)
        blocks.append(W_hh[l][p].T)
    wcat = np.concatenate(blocks, axis=1).astype(bf)                # [32,1152]
    b5 = (b_ih + b_hh)[:, p].astype(bf)                             # [5,128]
    oneh = np.zeros((L, F), np.float32)
    for l in range(L):
        oneh[l, l * BC:(l + 1) * BC] = 1.0
    oneh = oneh.astype(bf)
    svec = np.ones((128, 1), np.float32)
    svec[96:128] = 2.0
    fcw = np.concatenate([W_fc.reshape(H, 1), b_fc.reshape(1, 1)],
                         axis=0).astype(bf)                         # [33,1]
    ident = np.eye(BC, dtype=ml_dtypes.float8_e4m3)
    return {"w0": w0, "wcat": wcat, "b5": b5, "oneh": oneh, "svec": svec,
            "fcw": fcw, "ident": ident}


# ---------------- public entry point ----------------
N_CORES = 8
_NC_CACHE = {}


def _get_nc():
    if "nc" not in _NC_CACHE:
        _NC_CACHE["nc"] = build()
    return _NC_CACHE["nc"]


def _get_runner():
    """Cached jitted SPMD executor (avoids per-call re-trace/concat)."""
    if "runner" in _NC_CACHE:
        return _NC_CACHE["runner"]
    import jax
    from jax.sharding import Mesh, PartitionSpec, NamedSharding
    from jax.experimental.shard_map import shard_map
    from concourse.bass2jax import (_bass_exec_p, install_neuronx_cc_hook,
                                    partition_id_tensor)
    nc = _get_nc()
    install_neuronx_cc_hook()
    partition_name = (nc.partition_id_tensor.name
                      if nc.partition_id_tensor else None)
    in_names, out_names, out_avals, zero_outs = [], [], [], []
    for alloc in nc.m.functions[0].allocations:
        if not isinstance(alloc, mybir.MemoryLocationSet):
            continue
        name = alloc.memorylocations[0].name
        if alloc.kind == "ExternalInput":
            if name != partition_name:
                in_names.append(name)
        elif alloc.kind == "ExternalOutput":
            out_names.append(name)
            shape = tuple(alloc.tensor_shape)
            dtype = mybir.dt.np(alloc.dtype)
            out_avals.append(jax.core.ShapedArray(shape, dtype))
            zero_outs.append(np.zeros(shape, dtype))
    n_params = len(in_names)
    n_outs = len(out_avals)
    all_names = list(in_names) + out_names + (
        [partition_name] if partition_name else [])

    def _body(*args):
        operands = list(args)
        if partition_name is not None:
            operands.append(partition_id_tensor())
        outs = _bass_exec_p.bind(
            *operands, out_avals=tuple(out_avals), in_names=tuple(all_names),
            out_names=tuple(out_names), lowering_input_output_aliases=(),
            sim_require_finite=True, sim_require_nnan=True, nc=nc)
        return tuple(outs)

    devices = jax.devices()[:N_CORES]
    mesh = Mesh(np.asarray(devices), ("core",))
    in_specs = (PartitionSpec("core"),) * (n_params + n_outs)
    out_specs = (PartitionSpec("core"),) * len(out_names)
    donate = tuple(range(n_params, n_params + n_outs))
    f = jax.jit(shard_map(_body, mesh=mesh, in_specs=in_specs,
                          out_specs=out_specs, check_rep=False),
                donate_argnums=donate, keep_unused=True)
    sh = NamedSharding(mesh, PartitionSpec("core"))
    runner = dict(f=f, devices=devices, sh=sh, in_names=in_names,
                  out_names=out_names, zero_outs=zero_outs, jax=jax)
    _NC_CACHE["runner"] = runner
    return runner


def kernel(x, W_ih0, W_ih_rest, W_hh, b_ih, b_hh, W_fc, b_fc):
    import ml_dtypes
    from concurrent.futures import ThreadPoolExecutor
    r = _get_runner()
    jax, devices, sh = r["jax"], r["devices"], r["sh"]
    w = prep_weights(np.asarray(W_ih0), np.asarray(W_ih_rest),
                     np.asarray(W_hh), np.asarray(b_ih), np.asarray(b_hh),
                     np.asarray(W_fc), np.asarray(b_fc))
    x = np.asarray(x).reshape(256, T * D)

    ex = ThreadPoolExecutor(N_CORES)
    xt_futs = []
    for c in range(N_CORES):
        xq = x[c * BC:(c + 1) * BC].astype(ml_dtypes.float8_e4m3)
        xt_futs.append(ex.submit(jax.device_put, xq, devices[c]))

    def put_global(name):
        if name == "xt":
            shards = [f.result() for f in xt_futs]
        else:
            a = w[name]
            shards = list(ex.map(
                lambda c: jax.device_put(a, devices[c]), range(N_CORES)))
        shape = (N_CORES * shards[0].shape[0],) + shards[0].shape[1:]
        return jax.make_array_from_single_device_arrays(shape, sh, shards)

    args = [put_global(nm) for nm in r["in_names"]]
    for z in r["zero_outs"]:
        shards = list(ex.map(
            lambda c, z=z: jax.device_put(z, devices[c]), range(N_CORES)))
        shape = (N_CORES * z.shape[0],) + z.shape[1:]
        args.append(jax.make_array_from_single_device_arrays(shape, sh, shards))
    out_arrs = r["f"](*args)
    out = np.asarray(out_arrs[0]).reshape(N_CORES, N, P, BC)
    ex.shutdown(wait=False)
    full = np.empty((N_CORES, BC, T), np.float32)
    for p in range(P):
        kp = 0 if p == 0 else K
        o_p = p * SEG + (K if p > 0 else 0)
        rng = N - kp
        full[:, :, o_p:o_p + rng] = out[:, kp:, p, :].transpose(0, 2, 1)
    return full.reshape(256, T)[:, :, None].astype(np.float32)


# revision 5
# speedup vs baseline: 1.1535x; 1.1535x over previous
"""Trainium2 Bass kernel for the 5-layer LSTM + FC head (nn_LstmMemoryPredict).

Data-parallel over 8 NeuronCores (batch 256 -> 32 per core). The T=2048
recurrence is split into P=8 overlapping segments per core (warmup K=32;
forget-gate decay makes the seam error ~1e-6). The 8 segment-chains run
round-robin, hiding the ~2.7us serial LSTM-cell dependency chain behind
engine throughput. Per chain-step a 5-layer wavefront computes all gates
with one 128-partition sigmoid (gate order [f,i,o,g]; tanh via sigma(2x)
and a per-partition ACT scale vector). x ships as fp8_e4m3, transposed
on-device by the PE. Chains are paired (CG=2) per instruction group to halve per-instruction
overheads. Self-contained.
"""
import sys
sys.path.insert(0, "/opt/trn_rl_repo")

import numpy as np
import concourse.bass as bass
import concourse.bacc as bacc
import concourse.mybir as mybir
from concourse.tile import TileContext
from concourse.mybir import AluOpType, ActivationFunctionType

F32 = mybir.dt.float32
F16 = mybir.dt.float16
BF16 = mybir.dt.bfloat16
F8 = mybir.dt.float8e4

H, L, D = 32, 5, 64
BC = 32                  # batch per core
F = L * BC               # 160: free size of one chain step
PRO = L - 1              # wavefront warm-up steps
T = 2048
P = 8                    # recurrence segments (parallel chains)
CG = 2                   # chains per instruction group
G = P // CG              # instruction groups (4)
FG = CG * F              # free size per group instr (320)
K = 32                   # segment state warm-up steps
N = (T + (P - 1) * K) // P   # 284 main rounds
RB = 4                   # rounds per For_i body
NBODY = N // RB          # 71 bodies
SEG = N - K              # 252: t0 stride between chains
# torch gate order (i,f,g,o) -> kernel order (f,i,o,g)
GATE_PERM = np.r_[32:64, 0:32, 96:128, 64:96]


def build(unroll=False):
    nc = bacc.Bacc(None, target_bir_lowering=False, debug=False)

    xt_d = nc.dram_tensor("xt", [BC, T * D], F8, kind="ExternalInput")
    w0_d = nc.dram_tensor("w0", [D, 128], BF16, kind="ExternalInput")
    wcat_d = nc.dram_tensor("wcat", [H, 9 * 128], BF16, kind="ExternalInput")
    b5_d = nc.dram_tensor("b5", [L, 128], BF16, kind="ExternalInput")
    oneh_d = nc.dram_tensor("oneh", [L, CG * F], BF16, kind="ExternalInput")
    svec_d = nc.dram_tensor("svec", [128, 1], F32, kind="ExternalInput")
    fcw_d = nc.dram_tensor("fcw", [H + 1, 1], BF16, kind="ExternalInput")
    ident_d = nc.dram_tensor("ident", [BC, BC], F8, kind="ExternalInput")
    out_d = nc.dram_tensor("out", [1, N * P * BC], F32, kind="ExternalOutput")

    with TileContext(nc) as tc:
        with (
            tc.tile_pool(name="wpool", bufs=1) as wpool,
            tc.tile_pool(name="state", bufs=1) as state,
            tc.tile_pool(name="xpool", bufs=3) as xpool,
            tc.tile_pool(name="xsbp", bufs=3) as xsbp,
            tc.tile_pool(name="spool", bufs=6) as spool,
            tc.tile_pool(name="opool", bufs=3) as opool,
            tc.tile_pool(name="zpool", bufs=4, space="PSUM") as zpool,
            tc.tile_pool(name="xtp", bufs=2, space="PSUM") as xtp,
            tc.tile_pool(name="fcp", bufs=2, space="PSUM") as fcp,
        ):
            # ---- constants / weights ----
            w0 = wpool.tile([D, 128], BF16)
            wcat = wpool.tile([H, 9 * 128], BF16)
            b5 = wpool.tile([L, 128], BF16)
            oneh = wpool.tile([L, CG * F], BF16)
            svec = wpool.tile([128, 1], F32)
            fcw = wpool.tile([H + 1, 1], BF16)
            ident = wpool.tile([BC, BC], F8)
            nc.sync.dma_start(w0[:], w0_d[:, :])
            nc.sync.dma_start(wcat[:], wcat_d[:, :])
            nc.sync.dma_start(b5[:], b5_d[:, :])
            nc.sync.dma_start(oneh[:], oneh_d[:, :])
            nc.sync.dma_start(svec[:], svec_d[:, :])
            nc.sync.dma_start(fcw[:], fcw_d[:, :])
            nc.sync.dma_start(ident[:], ident_d[:, :])

            # ---- persistent state (per chain) ----
            hst = [state.tile([H + 1, FG], BF16, tag=f"hst{g}",
                              name=f"hst{g}") for g in range(G)]
            cst = [state.tile([H, FG], F16, tag=f"cst{g}", name=f"cst{g}")
                   for g in range(G)]
            for g in range(G):
                nc.gpsimd.memset(hst[g][:], 0.0)
                nc.gpsimd.memset(hst[g][H:H + 1, :], 1.0)
                nc.gpsimd.memset(cst[g][:], 0.0)

            def wh(l):
                return wcat[:, (2 * l) * 128:(2 * l + 1) * 128]

            def wx(l):
                return wcat[:, (2 * l - 1) * 128:(2 * l) * 128]

            def lwin(ap, l, rows=H, width=BC, base=0):
                # [rows, CG, width] windows at col l*width within each chain
                return ap[0:rows, :].rearrange(
                    "a (c r) -> a c r", c=CG)[:, :, base + l * width:
                                              base + l * width + width]

            def emit_step(g, x_mv, fc_ps=None, fc_col=0, mask_from=None):
                zb = zpool.tile([128, FG], F32, tag="zb", name="zb")
                nc.tensor.matmul(zb[:, :], b5[:], oneh[:], start=True,
                                 stop=False, skip_group_check=True)
                nc.tensor.matmul(lwin(zb, 0, rows=128), w0[:], x_mv,
                                 start=False, stop=False,
                                 skip_group_check=True)
                for l in range(L):
                    nc.tensor.matmul(lwin(zb, l, rows=128), wh(l),
                                     lwin(hst[g], l), start=False, stop=False,
                                     skip_group_check=True)
                for l in range(1, L):
                    nc.tensor.matmul(lwin(zb, l, rows=128), wx(l),
                                     lwin(hst[g], l - 1), start=False,
                                     stop=(l == L - 1), skip_group_check=True)
                sg = spool.tile([128, FG], F16, tag="sg", name="sg")
                nc.scalar.activation(sg[:, :], zb[:, :],
                                     ActivationFunctionType.Sigmoid,
                                     scale=svec[:, 0:1])
                # cell: g' = 2*sig(2zg)-1 (base 32); c' = f*c + i*g'
                a64 = spool.tile([2 * H, FG], F16, tag="a64", name="a64")
                nc.vector.tensor_scalar(a64[H:2 * H, :], sg[96:128, :],
                                        2.0, -1.0, AluOpType.mult,
                                        AluOpType.add)
                # f*c off the critical path -> gpsimd
                nc.gpsimd.tensor_tensor(a64[0:H, :], sg[0:H, :],
                                        cst[g][:, :], AluOpType.mult)
                p2 = spool.tile([H, FG], F16, tag="p2", name="p2")
                nc.vector.tensor_tensor(p2[:, :], sg[H:2 * H, :],
                                        a64[H:2 * H, :], AluOpType.mult)
                nc.vector.tensor_tensor(cst[g][:, :], a64[0:H, :], p2[:, :],
                                        AluOpType.add)
                th = spool.tile([3 * H, FG], F16, tag="th", name="th")
                nc.scalar.activation(th[2 * H:3 * H, :], cst[g][:, :],
                                     ActivationFunctionType.Tanh)
                nc.vector.tensor_tensor(hst[g][0:H, :], sg[64:96, :],
                                        th[2 * H:3 * H, :], AluOpType.mult)
                if fc_ps is not None:
                    nc.tensor.matmul(
                        fc_ps[:, fc_col:fc_col + CG * BC],
                        fcw[:], lwin(hst[g], L - 1, rows=H + 1),
                        start=True, stop=True, skip_group_check=True)
                if mask_from is not None:
                    for c in range(CG):
                        nc.gpsimd.memset(
                            hst[g][0:H, c * F + mask_from * BC:(c + 1) * F],
                            0.0)
                        nc.gpsimd.memset(
                            cst[g][:, c * F + mask_from * BC:(c + 1) * F],
                            0.0)

            def load_x(off_expr, nsteps, skip_chains=()):
                """DMA x for all P chains for `nsteps` rounds starting at
                x-time (t0_p + off) and transpose to [D, ...] layout.
                Returns xsb with col layout p*(nsteps*BC) + k*BC + b."""
                xb = xpool.tile([BC, P * nsteps * D], F8, name="xb")
                for p in skip_chains:
                    nc.gpsimd.memset(
                        xb[:, p * nsteps * D:(p + 1) * nsteps * D], 0.0)
                for p in range(P):
                    if p not in skip_chains:
                        nc.sync.dma_start(
                            xb[:, p * nsteps * D:(p + 1) * nsteps * D],
                            xt_d[:, bass.ds(p * SEG * D + off_expr,
                                            nsteps * D)])
                xt_ps = xtp.tile([D, P * nsteps * BC * 2], F8, tag="xt",
                                 name="xt_ps")
                for p in range(P):
                    for k in range(nsteps):
                        j = p * nsteps + k
                        nc.tensor.transpose(
                            xt_ps[:, 2 * j * BC:2 * (j + 1) * BC:2],
                            xb[:, j * D:(j + 1) * D], ident[:])
                xsb = xsbp.tile([D, P * nsteps * BC], BF16, name="xsb")
                nc.vector.tensor_copy(xsb[:, :],
                                      xt_ps[:, 0:2 * P * nsteps * BC:2])
                return xsb

            # ---- prologue: wavefront warm-up, rounds w=0..3 ----
            xsb_pro = load_x(0, PRO)
            for w in range(PRO):
                for g in range(G):
                    xg = xsb_pro[:, g * (CG * PRO * BC):
                                 (g + 1) * (CG * PRO * BC)].rearrange(
                        "a (c r) -> a c r", c=CG)[:, :, w * BC:(w + 1) * BC]
                    emit_step(g, xg, mask_from=w + 1)

            # ---- main loop: NBODY bodies of RB rounds ----
            def emit_body(i, last=False):
                # x for rounds i*RB .. i*RB+RB-1: x-time t0_p + PRO + i*RB + k
                if last:
                    xsb = load_x((PRO + i * RB) * D, RB, skip_chains=(P - 1,))
                else:
                    xsb = load_x((PRO + i * RB) * D
                                 if isinstance(i, int) else
                                 i * (RB * D) + PRO * D, RB)
                for half in range(RB // 2):
                    fc_ps = fcp.tile([1, 2 * P * BC], F32, tag="fc",
                                     name="fc_ps")
                    for k2 in range(2):
                        k = half * 2 + k2
                        for g in range(G):
                            xg = xsb[:, g * (CG * RB * BC):
                                     (g + 1) * (CG * RB * BC)].rearrange(
                                "a (c r) -> a c r",
                                c=CG)[:, :, k * BC:(k + 1) * BC]
                            emit_step(g, xg, fc_ps=fc_ps,
                                      fc_col=(k2 * P + g * CG) * BC)
                    ost = opool.tile([1, 2 * P * BC], F32, tag="ost",
                                     name="ost")
                    nc.vector.tensor_copy(ost[:], fc_ps[:, :])
                    if isinstance(i, int):
                        off = (i * RB + half * 2) * (P * BC)
                    else:
                        off = i * (RB * P * BC) + half * 2 * (P * BC)
                    nc.sync.dma_start(
                        out_d[:, bass.ds(off, 2 * P * BC)], ost[:])

            if unroll:
                for i in range(NBODY - 1):
                    emit_body(i)
            else:
                with tc.For_i(0, NBODY - 1) as i:
                    emit_body(i)
            emit_body(NBODY - 1, last=True)

    nc.compile()
    return nc


# ---------------- host-side packing ----------------

def prep_weights(W_ih0, W_ih_rest, W_hh, b_ih, b_hh, W_fc, b_fc):
    import ml_dtypes
    bf = ml_dtypes.bfloat16
    p = GATE_PERM
    w0 = np.ascontiguousarray(W_ih0[p].T).astype(bf)                # [64,128]
    blocks = [W_hh[0][p].T]
    for l in range(1, L):
        blocks.append(W_ih_rest[l - 1][p].T)
        blocks.append(W_hh[l][p].T)
    wcat = np.concatenate(blocks, axis=1).astype(bf)                # [32,1152]
    b5 = (b_ih + b_hh)[:, p].astype(bf)                             # [5,128]
    oneh = np.zeros((L, F), np.float32)
    for l in range(L):
        oneh[l, l * BC:(l + 1) * BC] = 1.0
    oneh = np.tile(oneh, (1, CG)).astype(bf)
    svec = np.ones((128, 1), np.float32)
    svec[96:128] = 2.0
    fcw = np.concatenate([W_fc.reshape(H, 1), b_fc.reshape(1, 1)],
                         axis=0).astype(bf)                         # [33,1]
    ident = np.eye(BC, dtype=ml_dtypes.float8_e4m3)
    return {"w0": w0, "wcat": wcat, "b5": b5, "oneh": oneh, "svec": svec,
            "fcw": fcw, "ident": ident}


# ---------------- public entry point ----------------
N_CORES = 8
_NC_CACHE = {}


def _get_nc():
    if "nc" not in _NC_CACHE:
        _NC_CACHE["nc"] = build()
    return _NC_CACHE["nc"]


def _get_runner():
    """Cached jitted SPMD executor (avoids per-call re-trace/concat)."""
    if "runner" in _NC_CACHE:
        return _NC_CACHE["runner"]
    import jax
    from jax.sharding import Mesh, PartitionSpec, NamedSharding
    from jax.experimental.shard_map import shard_map
    from concourse.bass2jax import (_bass_exec_p, install_neuronx_cc_hook,
                                    partition_id_tensor)
    nc = _get_nc()
    install_neuronx_cc_hook()
    partition_name = (nc.partition_id_tensor.name
                      if nc.partition_id_tensor else None)
    in_names, out_names, out_avals, zero_outs = [], [], [], []
    for alloc in nc.m.functions[0].allocations:
        if not isinstance(alloc, mybir.MemoryLocationSet):
            continue
        name = alloc.memorylocations[0].name
        if alloc.kind == "ExternalInput":
            if name != partition_name:
                in_names.append(name)
        elif alloc.kind == "ExternalOutput":
            out_names.append(name)
            shape = tuple(alloc.tensor_shape)
            dtype = mybir.dt.np(alloc.dtype)
            out_avals.append(jax.core.ShapedArray(shape, dtype))
            zero_outs.append(np.zeros(shape, dtype))
    n_params = len(in_names)
    n_outs = len(out_avals)
    all_names = list(in_names) + out_names + (
        [partition_name] if partition_name else [])

    def _body(*args):
        operands = list(args)
        if partition_name is not None:
            operands.append(partition_id_tensor())
        outs = _bass_exec_p.bind(
            *operands, out_avals=tuple(out_avals), in_names=tuple(all_names),
            out_names=tuple(out_names), lowering_input_output_aliases=(),
            sim_require_finite=True, sim_require_nnan=True, nc=nc)
        return tuple(outs)

    devices = jax.devices()[:N_CORES]
    mesh = Mesh(np.asarray(devices), ("core",))
    in_specs = (PartitionSpec("core"),) * (n_params + n_outs)
    out_specs = (PartitionSpec("core"),) * len(out_names)
    donate = tuple(range(n_params, n_params + n_outs))
    f = jax.jit(shard_map(_body, mesh=mesh, in_specs=in_specs,
                          out_specs=out_specs, check_rep=False),
                donate_argnums=donate, keep_unused=True)
    sh = NamedSharding(mesh, PartitionSpec("core"))
    runner = dict(f=f, devices=devices, sh=sh, in_names=in_names,
                  out_names=out_names, zero_outs=zero_outs, jax=jax)
    _NC_CACHE["runner"] = runner
    return runner


def kernel(x, W_ih0, W_ih_rest, W_hh, b_ih, b_hh, W_fc, b_fc):
    import ml_dtypes
    from concurrent.futures import ThreadPoolExecutor
    r = _get_runner()
    jax, devices, sh = r["jax"], r["devices"], r["sh"]
    w = prep_weights(np.asarray(W_ih0), np.asarray(W_ih_rest),
                     np.asarray(W_hh), np.asarray(b_ih), np.asarray(b_hh),
                     np.asarray(W_fc), np.asarray(b_fc))
    x = np.asarray(x).reshape(256, T * D)

    ex = ThreadPoolExecutor(N_CORES)
    xt_futs = []
    for c in range(N_CORES):
        xq = x[c * BC:(c + 1) * BC].astype(ml_dtypes.float8_e4m3)
        xt_futs.append(ex.submit(jax.device_put, xq, devices[c]))

    def put_global(name):
        if name == "xt":
            shards = [f.result() for f in xt_futs]
        else:
            a = w[name]
            shards = list(ex.map(
                lambda c: jax.device_put(a, devices[c]), range(N_CORES)))
        shape = (N_CORES * shards[0].shape[0],) + shards[0].shape[1:]
        return jax.make_array_from_single_device_arrays(shape, sh, shards)

    args = [put_global(nm) for nm in r["in_names"]]
    for z in r["zero_outs"]:
        shards = list(ex.map(
            lambda c, z=z: jax.device_put(z, devices[c]), range(N_CORES)))
        shape = (N_CORES * z.shape[0],) + z.shape[1:]
        args.append(jax.make_array_from_single_device_arrays(shape, sh, shards))
    out_arrs = r["f"](*args)
    out = np.asarray(out_arrs[0]).reshape(N_CORES, N, P, BC)
    ex.shutdown(wait=False)
    full = np.empty((N_CORES, BC, T), np.float32)
    for p in range(P):
        kp = 0 if p == 0 else K
        o_p = p * SEG + (K if p > 0 else 0)
        rng = N - kp
        full[:, :, o_p:o_p + rng] = out[:, kp:, p, :].transpose(0, 2, 1)
    return full.reshape(256, T)[:, :, None].astype(np.float32)
